# revision 35
# baseline (speedup 1.0000x reference)
"""Self-contained TRN2 Bass kernel for nn_MultiHeadAttention_77833397338481.

kernel(**inputs) takes the FULL unsharded inputs (Q, K, V [2,1024,1024],
Wq/Wk/Wv/Wo [1024,1024], biases [1024]) and returns the FULL output
[2, 1024, 1024]. 8 NeuronCores = batch(2) x head-group(4); fp32r matmuls.

v2: col-tile-packed ctx matmuls (4 heads concurrent), dense ctx +
dense output projection, ACT-engine projection evacuations, batched
softmax-normalization chain, PSUM->DRAM output DMA.
"""

import numpy as np

import concourse.bacc as bacc
import concourse.mybir as mybir
import concourse.tile as tile

F32 = mybir.dt.float32
F32R = mybir.dt.float32r
I32 = mybir.dt.int32
AF = mybir.ActivationFunctionType
ALU = mybir.AluOpType

D = 1024
S = 1024
B = 2
E = 16
NHQ = 4     # head-groups (j) per quadrant tile
NQUAD = 4   # quadrant tiles (t) per core
HPC = NHQ * NQUAD  # 16 heads per core
VW = HPC * 17  # 272 = 16 heads x (16 dims + ones column)
VWP = VW + 16  # padded so the 32-wide ctx lhsT slice of the last head exists
SCALE = 1.0 / 32.0
ND = D // 128
NS = S // 128

# DVE dual-phase Schraudolph exp offload: which i-blocks (of 8 per (n,t))
# are exponentiated on the vector engine instead of ACT.
OFFLOAD_I = ()

_L2E = 1.4426950408889634  # log2(e)
_C_NUDGE = 0.054
_EXP_A = (1 << 23) * _L2E * SCALE          # folds the 1/32 score scale
_EXP_B1 = (127.0 - _C_NUDGE - 1.0) * (1 << 23)  # phase 0, exponent-halved
_EXP_B2 = _EXP_B1 + (1 << 22)                   # phase +0.5
_EXP_W2 = 2.0 ** -1.5


def round_fp32r(x):
    u = np.ascontiguousarray(x, np.float32).view(np.uint32)
    r = ((u.astype(np.uint64) + 0x800) & 0xFFFFF000).astype(np.uint32)
    return r.view(np.float32)


def build_nc():
    nc = bacc.Bacc("TRN2", target_bir_lowering=False, debug=False, num_devices=8)

    xt_q = nc.dram_tensor("xt_q", [D, S], F32R, kind="ExternalInput")
    xt_k = nc.dram_tensor("xt_k", [D, S], F32R, kind="ExternalInput")
    xt_v = nc.dram_tensor("xt_v", [D, S], F32R, kind="ExternalInput")
    wqt = nc.dram_tensor("wqt", [D, 512], F32R, kind="ExternalInput")
    wkt = nc.dram_tensor("wkt", [D, 512], F32R, kind="ExternalInput")
    wvt = nc.dram_tensor("wvt", [D, VW], F32R, kind="ExternalInput")
    wot = nc.dram_tensor("wot", [512, D], F32R, kind="ExternalInput")
    ind_all = nc.dram_tensor("ind_all", [8, 256], F32R, kind="ExternalInput")
    bqp = nc.dram_tensor("bqp", [128, NQUAD], F32, kind="ExternalInput")
    bkp = nc.dram_tensor("bkp", [128, NQUAD], F32, kind="ExternalInput")
    out_d = nc.dram_tensor("out_part", [S, D], F32, kind="ExternalOutput")

    with tile.TileContext(nc) as tc:
        with (
            tc.tile_pool(name="persist", bufs=1) as pp,
            tc.tile_pool(name="proj", bufs=1) as jp,
            tc.tile_pool(name="attn", bufs=1) as ap_,
            tc.tile_pool(name="psum", space="PSUM", bufs=1) as ps,
        ):
            # --- warm up the exp table ASAP ---
            dummy = pp.tile([1, 8], F32, name="dummy")
            nc.vector.memset(dummy, 0.0)
            dummy2 = pp.tile([1, 8], F32, name="dummy2")
            nc.scalar.activation(dummy2, dummy, AF.Exp)

            # --- constants ---
            ind_sb = pp.tile([8, 256], F32R, name="ind_sb")
            nc.sync.dma_start(out=ind_sb, in_=ind_all[:])
            bq_sb = pp.tile([128, NQUAD], F32, name="bq_sb")
            nc.sync.dma_start(out=bq_sb, in_=bqp[:])
            bk_sb = pp.tile([128, NQUAD], F32, name="bk_sb")
            nc.sync.dma_start(out=bk_sb, in_=bkp[:])

            # --- persistent activations ---
            qt_sb = [pp.tile([128, S], F32R, name=f"qt{t}") for t in range(NQUAD)]
            kt_sb = [pp.tile([128, S], F32R, name=f"kt{t}") for t in range(NQUAD)]
            va_sb = [pp.tile([128, VWP], F32R, name=f"va{s}") for s in range(NS)]
            # ones columns of va (denominator trick), set once via ACT
            # strided copies (a DMA scatter here floods the rings with
            # 16k one-element packets and stalls the input loads)
            ones16 = pp.tile([128, HPC], F32, name="ones16")
            nc.vector.memset(ones16, 1.0)
            for s in range(NS):
                nc.vector.memset(va_sb[s][:, VW:VWP].bitcast(F32), 0.0)
                ones_cols = va_sb[s][:, 0:VW].rearrange("p (m c) -> p m c", c=17)[
                    :, :, 16:17
                ]
                nc.scalar.copy(ones_cols, ones16[:, :].unsqueeze(2))
            ctxp = [pp.tile([128, S], F32R, name=f"ctxp{t}") for t in range(NQUAD)]
            for t in range(NQUAD):
                nc.vector.memset(ctxp[t][:, :].bitcast(F32), 0.0)
            wot_sb = [
                ap_.tile([128, D], F32R, name=f"wot{t}", tag="wot", bufs=NQUAD)
                for t in range(NQUAD)
            ]

            # ============ projections (d-streamed, one xt pass each) ========
            def stream_proj(xt_dram, w_dram, wv_width, emit_mms, emit_evacs, name):
                """One pass over d: 8 accumulation groups (4 in the sc tile's
                512-wide slices + 4 in cx-tag tiles)."""
                scg_a = ps.tile([128, 1024], F32, name=f"scga_{name}", tag="sc", bufs=2)
                scg_b = ps.tile([128, 1024], F32, name=f"scgb_{name}", tag="sc", bufs=2)
                scg = (scg_a, scg_b)
                ctxg = [
                    ps.tile([128, 512], F32, name=f"cg_{name}{g}", tag="cx", bufs=4)
                    for g in range(4)
                ]
                xts, ws = [], []
                for d in range(ND):
                    xv = jp.tile(
                        [128, S], F32R, name=f"x_{name}{d}", tag="xt", bufs=8
                    )
                    nc.sync.dma_start(out=xv, in_=xt_dram[128 * d : 128 * (d + 1), :])
                    xts.append(xv)
                    wv = jp.tile(
                        [128, wv_width], F32R, name=f"w_{name}{d}", tag="wp", bufs=6
                    )
                    nc.sync.dma_start(out=wv, in_=w_dram[128 * d : 128 * (d + 1), :])
                    ws.append(wv)
                for d in range(ND):
                    emit_mms(d, xts[d], ws[d], scg, ctxg)
                emit_evacs(scg, ctxg)

            def v_mms(d, xv, wv, scg, ctxg):
                for s in range(NS):
                    dst = (
                        scg[s // 2][:, 512 * (s % 2) : 512 * (s % 2) + VW]
                        if s < 4
                        else ctxg[s - 4][:, 0:VW]
                    )
                    nc.tensor.matmul(
                        dst,
                        xv[:, 128 * s : 128 * (s + 1)],
                        wv,
                        start=(d == 0),
                        stop=(d == ND - 1),
                    )

            def v_evacs(scg, ctxg):
                # copy only the 16 data columns of each 17-wide head slot
                # (ones columns pre-set); ACT engine is idle in this phase
                for s in range(NS):
                    src = (
                        scg[s // 2][:, 512 * (s % 2) : 512 * (s % 2) + VW]
                        if s < 4
                        else ctxg[s - 4][:, 0:VW]
                    )
                    src3 = src.rearrange("p (m c) -> p m c", c=17)
                    dst3 = va_sb[s][:, 0:VW].rearrange("p (m c) -> p m c", c=17)
                    # DVE (not ACT): V runs last, concurrent with attention exps
                    nc.vector.tensor_copy(dst3[:, :, 0:16], src3[:, :, 0:16])

            def qk_mms_factory(which):
                def mms(d, xv, wv, scg, ctxg):
                    for t in range(NQUAD):
                        for h in range(2):
                            g = 2 * t + h
                            dst = (
                                scg[g // 2][:, 512 * (g % 2) : 512 * (g % 2 + 1)]
                                if g < 4
                                else ctxg[g - 4]
                            )
                            nc.tensor.matmul(
                                dst,
                                wv[:, 128 * t : 128 * (t + 1)],
                                xv[:, 512 * h : 512 * (h + 1)],
                                start=(d == 0),
                                stop=(d == ND - 1),
                            )

                return mms

            def qk_evacs_factory(dst_tiles, bias):
                def evacs(scg, ctxg):
                    for t in range(NQUAD):
                        for h in range(2):
                            g = 2 * t + h
                            src = (
                                scg[g // 2][:, 512 * (g % 2) : 512 * (g % 2 + 1)]
                                if g < 4
                                else ctxg[g - 4]
                            )
                            nc.scalar.activation(
                                dst_tiles[t][:, 512 * h : 512 * (h + 1)],
                                src,
                                AF.Identity,
                                bias=bias[:, t : t + 1],
                            )

                return evacs

            stream_proj(xt_q, wqt, 512, qk_mms_factory("q"), qk_evacs_factory(qt_sb, bq_sb), "q")
            stream_proj(xt_k, wkt, 512, qk_mms_factory("k"), qk_evacs_factory(kt_sb, bk_sb), "k")
            stream_proj(xt_v, wvt, VW, v_mms, v_evacs, "v")

            for t in range(NQUAD):
                nc.sync.dma_start(out=wot_sb[t], in_=wot[128 * t : 128 * (t + 1), :])

            # ================= attention (n-outer) =================
            def outproj_group(m, evac_act=False):
                # both 512-col halves land in one og tile so the output DMA
                # is a single contiguous 512KB transfer (a per-half DMA is
                # 128 strided 2KB packets and drains ~10us past last compute)
                og = ap_.tile([128, 1024], F32, name=f"og{m}", tag="og", bufs=4)
                for dc in range(2):
                    po = ps.tile([128, 512], F32, name=f"po{m}{dc}", tag="cx", bufs=4)
                    for t_ in range(NQUAD):
                        nc.tensor.matmul(
                            po,
                            ctxp[t_][:, 128 * m : 128 * (m + 1)],
                            wot_sb[t_][:, 512 * dc : 512 * (dc + 1)],
                            start=(t_ == 0),
                            stop=(t_ == NQUAD - 1),
                        )
                    if evac_act:
                        nc.scalar.copy(og[:, 512 * dc : 512 * (dc + 1)], po)
                    else:
                        nc.vector.tensor_copy(og[:, 512 * dc : 512 * (dc + 1)], po)
                for rq in range(4):
                    r0 = 128 * m + 32 * rq
                    nc.sync.dma_start(
                        out=out_d[r0 : r0 + 32, :], in_=og[32 * rq : 32 * rq + 32, :]
                    )

            for n in range(2):  # sq chunks of 512
                stages = {}
                den_h = {}
                for t in range(NQUAD):
                    if t % 2 == 0:
                        den_h[t // 2] = ap_.tile(
                            [8, 512], F32, name=f"den{n}{t//2}", tag="den", bufs=2
                        )
                    den_n = den_h[t // 2]
                    ctx_t = [
                        ps.tile([17, 512], F32, name=f"ctx{t}{n}{j}", tag="cx", bufs=4)
                        for j in range(NHQ)
                    ]
                    ex_tiles = []
                    for i in range(NS):  # sk blocks of 128
                        sc_a = ps.tile(
                            [128, 1024], F32, name=f"sca{t}{n}{i}", tag="sc", bufs=2
                        )
                        sc_b = ps.tile(
                            [128, 1024], F32, name=f"scb{t}{n}{i}", tag="sc", bufs=2
                        )
                        ex = ap_.tile(
                            [128, 2048], F32R, name=f"ex{t}{n}{i}", tag="ex", bufs=3
                        )
                        for j in range(NHQ):
                            sch = sc_a if j < 2 else sc_b
                            nc.tensor.matmul(
                                sch[:, 512 * (j % 2) : 512 * (j % 2 + 1)],
                                kt_sb[t][32 * j : 32 * j + 32, 128 * i : 128 * (i + 1)],
                                qt_sb[t][32 * j : 32 * j + 32, 512 * n : 512 * (n + 1)],
                                start=True,
                                stop=True,
                                tile_position=(32 * j, 0),
                            )
                        nc.scalar.activation(ex[:, 0:1024], sc_a, AF.Exp, scale=SCALE)
                        nc.scalar.activation(
                            ex[:, 1024:2048], sc_b, AF.Exp, scale=SCALE
                        )
                        ex_tiles.append(ex)
                        if i >= 1:
                            for j in range(NHQ):
                                mq = 17 * (NHQ * t + j)
                                nc.tensor.matmul(
                                    ctx_t[j],
                                    va_sb[i - 1][:, mq : mq + 17],
                                    ex_tiles[i - 1][:, 512 * j : 512 * (j + 1)],
                                    start=(i - 1 == 0),
                                    stop=False,
                                )
                    for j in range(NHQ):
                        mq = 17 * (NHQ * t + j)
                        nc.tensor.matmul(
                            ctx_t[j],
                            va_sb[NS - 1][:, mq : mq + 17],
                            ex_tiles[NS - 1][:, 512 * j : 512 * (j + 1)],
                            start=False,
                            stop=True,
                        )

                    # stage ctx + gather denominators (ones rows at 32j+16)
                    st = ap_.tile(
                        [128, 512], F32, name=f"st{t}{n}", tag="stage", bufs=6
                    )
                    for j in range(NHQ):
                        nc.vector.tensor_copy(
                            st[32 * j : 32 * j + 17, :], ctx_t[j][0:17, :]
                        )
                        m8 = NHQ * (t % 2) + j
                        nc.sync.dma_start(
                            out=den_n[m8 : m8 + 1, :],
                            in_=st[32 * j + 16 : 32 * j + 17, :],
                        )
                    stages[t] = st

                    # interleave first-half output projections into n=1
                    if n == 1 and t >= 2:
                        for m_ in (2 * (t - 2), 2 * (t - 2) + 1):
                            outproj_group(m_)

                    if n == 1 and t == 3:
                        # PE warm-up burst overlapping the final normalize's
                        # DVE chain, so the tail outproj runs at full clock
                        dum_ps = ps.tile(
                            [128, 512], F32, name="dum_ps", tag="sc", bufs=2
                        )
                        for w_i in range(18):
                            nc.tensor.matmul(
                                dum_ps,
                                kt_sb[w_i % 4][:, 0:128],
                                qt_sb[w_i % 4][:, 0:512],
                                start=True,
                                stop=True,
                                skip_group_check=True,
                            )
                        nc.vector.tensor_copy(dummy, dum_ps[0:1, 0:8])

                    if t % 2 == 1:
                        # normalize the half's 8 heads
                        h = t // 2
                        with tc.high_priority(offset=-160):
                            recip = ap_.tile(
                                [8, 512], F32, name=f"rc{n}{h}", tag="recip", bufs=2
                            )
                            scratch = ap_.tile(
                                [8, 512], F32, name=f"rs{n}{h}", tag="recip", bufs=2
                            )
                            nc.vector.reciprocal_approx_accurate(
                                recip, den_h[h], scratch
                            )
                            recipr = ap_.tile(
                                [8, 512], F32R, name=f"rr{n}{h}", tag="recipr", bufs=2
                            )
                            nc.vector.tensor_copy(recipr, recip)
                            for t_ in (2 * h, 2 * h + 1):
                                rbw = ps.tile(
                                    [128, 512], F32, name=f"rbw{n}{t_}", tag="cx",
                                    bufs=4,
                                )
                                nc.tensor.matmul(
                                    rbw,
                                    ind_sb[:, 128 * (t_ % 2) : 128 * (t_ % 2 + 1)],
                                    recipr,
                                    start=True,
                                    stop=True,
                                )
                                # evacuate the PSUM bank fast, then normalize
                                # on the (otherwise idle) GPSIMD engine
                                rb = ap_.tile(
                                    [128, 512], F32, name=f"rb{n}{t_}", tag="rb",
                                    bufs=2,
                                )
                                nc.vector.tensor_copy(rb, rbw)
                                eng = nc.gpsimd if n == 0 else nc.vector
                                for j in range(NHQ):
                                    eng.tensor_tensor(
                                        ctxp[t_][
                                            32 * j : 32 * j + 16,
                                            512 * n : 512 * (n + 1),
                                        ],
                                        rb[32 * j : 32 * j + 16, :],
                                        stages[t_][32 * j : 32 * j + 16, :],
                                        ALU.mult,
                                    )

            # remaining output projections (second sq half)
            for m in range(4, 8):
                outproj_group(m, evac_act=True)

    nc.finalize()
    return nc


def prep_core_weights(g, Wq, bq, Wk, bk, Wv, Wo):
    C0 = 256 * g
    wqt = np.zeros((D, 512), np.float32)
    wkt = np.zeros((D, 512), np.float32)
    wvt = np.zeros((D, VW), np.float32)
    wot = np.zeros((512, D), np.float32)
    bqp = np.zeros((128, NQUAD), np.float32)
    bkp = np.zeros((128, NQUAD), np.float32)
    for t in range(NQUAD):
        for j in range(NHQ):
            src = C0 + 64 * t + 16 * j
            wqt[:, 128 * t + 32 * j : 128 * t + 32 * j + E] = Wq[src : src + E, :].T
            wkt[:, 128 * t + 32 * j : 128 * t + 32 * j + E] = Wk[src : src + E, :].T
            bqp[32 * j : 32 * j + E, t] = bq[src : src + E]
            bkp[32 * j : 32 * j + E, t] = bk[src : src + E]
            m = NHQ * t + j
            wvt[:, 17 * m : 17 * m + E] = Wv[src : src + E, :].T
            wot[128 * t + 32 * j : 128 * t + 32 * j + E, :] = Wo[:, src : src + E].T
    ind_np = np.zeros((8, 256), np.float32)
    for m8 in range(8):
        ind_np[m8, 128 * (m8 // 4) + 32 * (m8 % 4) : 128 * (m8 // 4) + 32 * (m8 % 4) + E] = 1.0
    return {
        "wqt": round_fp32r(wqt),
        "wkt": round_fp32r(wkt),
        "wvt": round_fp32r(wvt),
        "wot": round_fp32r(wot),
        "bqp": bqp,
        "bkp": bkp,
        "ind_all": round_fp32r(ind_np),
    }


def prep_in_maps(Q, K, V, Wq, bq, Wk, bk, Wv, Wo):
    group_w = [prep_core_weights(g, Wq, bq, Wk, bk, Wv, Wo) for g in range(4)]
    xt = []
    for b in range(B):
        xt.append(
            {
                "xt_q": round_fp32r(np.ascontiguousarray(Q[b].T)),
                "xt_k": round_fp32r(np.ascontiguousarray(K[b].T)),
                "xt_v": round_fp32r(np.ascontiguousarray(V[b].T)),
            }
        )
    in_maps = []
    for c in range(8):
        b, g = c // 4, c % 4
        m = dict(group_w[g])
        m.update(xt[b])
        in_maps.append(m)
    return in_maps


def assemble_output(results, bv, bo, Wo):
    # v-bias is folded here: sum_k attn = 1 exactly, so the missing
    # (v + bv) contribution is the constant vector Wo @ bv per position.
    const = (
        Wo.astype(np.float64) @ bv.astype(np.float64) + bo.astype(np.float64)
    )
    out = np.zeros((B, S, D), np.float32)
    for b in range(B):
        acc = np.zeros((S, D), np.float64)
        for g in range(4):
            acc += results[4 * b + g]["out_part"].astype(np.float64)
        out[b] = (acc + const).astype(np.float32)
    return out


_NC_CACHE = {}


def _get_nc():
    if "nc" not in _NC_CACHE:
        _NC_CACHE["nc"] = build_nc()
    return _NC_CACHE["nc"]


def kernel(Q, K, V, Wq, bq, Wk, bk, Wv, bv, Wo, bo):
    import time

    from concourse.bass_utils import run_bass_kernel_spmd

    nc = _get_nc()
    in_maps = prep_in_maps(
        np.asarray(Q, np.float32),
        np.asarray(K, np.float32),
        np.asarray(V, np.float32),
        np.asarray(Wq, np.float32),
        np.asarray(bq, np.float32),
        np.asarray(Wk, np.float32),
        np.asarray(bk, np.float32),
        np.asarray(Wv, np.float32),
        np.asarray(Wo, np.float32),
    )
    # Retries: a first execution after NEFF load occasionally hits a
    # transient NRT_EXEC_UNIT_UNRECOVERABLE; re-running recovers.
    last = None
    for attempt in range(3):
        try:
            res = run_bass_kernel_spmd(nc, in_maps, list(range(8)))
            return assemble_output(
                res.results,
                np.asarray(bv, np.float32),
                np.asarray(bo, np.float32),
                np.asarray(Wo, np.float32),
            )
        except Exception as e:
            last = e
            time.sleep(3)
    raise last


# revision 36
# speedup vs baseline: 1.0960x; 1.0960x over previous
"""Self-contained TRN2 Bass kernel for nn_MultiHeadAttention_77833397338481.

kernel(**inputs) takes the FULL unsharded inputs (Q, K, V [2,1024,1024],
Wq/Wk/Wv/Wo [1024,1024], biases [1024]) and returns the FULL output
[2, 1024, 1024]. 8 NeuronCores = batch(2) x head-group(4); fp32r matmuls.

v2: col-tile-packed ctx matmuls (4 heads concurrent), dense ctx +
dense output projection, ACT-engine projection evacuations, batched
softmax-normalization chain, PSUM->DRAM output DMA.
"""

import numpy as np

import concourse.bacc as bacc
import concourse.mybir as mybir
import concourse.tile as tile

F32 = mybir.dt.float32
F32R = mybir.dt.float32r
I32 = mybir.dt.int32
AF = mybir.ActivationFunctionType
ALU = mybir.AluOpType

D = 1024
S = 1024
B = 2
E = 16
NHQ = 4     # head-groups (j) per quadrant tile
NQUAD = 4   # quadrant tiles (t) per core
HPC = NHQ * NQUAD  # 16 heads per core
VW = HPC * 17  # 272 = 16 heads x (16 dims + ones column)
VWP = VW + 16  # padded so the 32-wide ctx lhsT slice of the last head exists
SCALE = 1.0 / 32.0
ND = D // 128
NS = S // 128

# DVE dual-phase Schraudolph exp offload: which i-blocks (of 8 per (n,t))
# are exponentiated on the vector engine instead of ACT.
OFFLOAD_I = ()

_L2E = 1.4426950408889634  # log2(e)
_C_NUDGE = 0.054
_EXP_A = (1 << 23) * _L2E * SCALE          # folds the 1/32 score scale
_EXP_B1 = (127.0 - _C_NUDGE - 1.0) * (1 << 23)  # phase 0, exponent-halved
_EXP_B2 = _EXP_B1 + (1 << 22)                   # phase +0.5
_EXP_W2 = 2.0 ** -1.5


def round_fp32r(x):
    u = np.ascontiguousarray(x, np.float32).view(np.uint32)
    r = ((u.astype(np.uint64) + 0x800) & 0xFFFFF000).astype(np.uint32)
    return r.view(np.float32)


def build_nc():
    nc = bacc.Bacc("TRN2", target_bir_lowering=False, debug=False, num_devices=8)

    xt_q = nc.dram_tensor("xt_q", [D, S], F32R, kind="ExternalInput")
    xt_k = nc.dram_tensor("xt_k", [D, S], F32R, kind="ExternalInput")
    xt_v = nc.dram_tensor("xt_v", [D, S], F32R, kind="ExternalInput")
    wqt = nc.dram_tensor("wqt", [D, 512], F32R, kind="ExternalInput")
    wkt = nc.dram_tensor("wkt", [D, 512], F32R, kind="ExternalInput")
    wvt = nc.dram_tensor("wvt", [D, VW], F32R, kind="ExternalInput")
    wot = nc.dram_tensor("wot", [512, D], F32R, kind="ExternalInput")
    ind_all = nc.dram_tensor("ind_all", [8, 256], F32R, kind="ExternalInput")
    bqp = nc.dram_tensor("bqp", [128, NQUAD], F32, kind="ExternalInput")
    bkp = nc.dram_tensor("bkp", [128, NQUAD], F32, kind="ExternalInput")
    out_d = nc.dram_tensor("out_part", [S, D], F32, kind="ExternalOutput")

    with tile.TileContext(nc) as tc:
        with (
            tc.tile_pool(name="persist", bufs=1) as pp,
            tc.tile_pool(name="proj", bufs=1) as jp,
            tc.tile_pool(name="attn", bufs=1) as ap_,
            tc.tile_pool(name="psum", space="PSUM", bufs=1) as ps,
        ):
            # --- warm up the exp table ASAP ---
            dummy = pp.tile([1, 8], F32, name="dummy")
            nc.vector.memset(dummy, 0.0)
            dummy2 = pp.tile([1, 8], F32, name="dummy2")
            nc.scalar.activation(dummy2, dummy, AF.Exp)

            # --- constants ---
            ind_sb = pp.tile([8, 256], F32R, name="ind_sb")
            nc.sync.dma_start(out=ind_sb, in_=ind_all[:])
            bq_sb = pp.tile([128, NQUAD], F32, name="bq_sb")
            nc.sync.dma_start(out=bq_sb, in_=bqp[:])
            bk_sb = pp.tile([128, NQUAD], F32, name="bk_sb")
            nc.sync.dma_start(out=bk_sb, in_=bkp[:])

            # --- persistent activations ---
            qt_sb = [pp.tile([128, S], F32R, name=f"qt{t}") for t in range(NQUAD)]
            kt_sb = [pp.tile([128, S], F32R, name=f"kt{t}") for t in range(NQUAD)]
            va_sb = [pp.tile([128, VWP], F32R, name=f"va{s}") for s in range(NS)]
            # ones columns of va (denominator trick), set once via ACT
            # strided copies (a DMA scatter here floods the rings with
            # 16k one-element packets and stalls the input loads)
            ones16 = pp.tile([128, HPC], F32, name="ones16")
            nc.vector.memset(ones16, 1.0)
            for s in range(NS):
                nc.vector.memset(va_sb[s][:, VW:VWP].bitcast(F32), 0.0)
                ones_cols = va_sb[s][:, 0:VW].rearrange("p (m c) -> p m c", c=17)[
                    :, :, 16:17
                ]
                nc.scalar.copy(ones_cols, ones16[:, :].unsqueeze(2))
            ctxp = [pp.tile([128, S], F32R, name=f"ctxp{t}") for t in range(NQUAD)]
            for t in range(NQUAD):
                nc.vector.memset(ctxp[t][:, :].bitcast(F32), 0.0)
            wot_sb = [
                ap_.tile([128, D], F32R, name=f"wot{t}", tag="wot", bufs=NQUAD)
                for t in range(NQUAD)
            ]

            # ============ projections (d-streamed, one xt pass each) ========
            def stream_proj(xt_dram, w_dram, wv_width, emit_mms, emit_evacs, name):
                """One pass over d: 8 accumulation groups (4 in the sc tile's
                512-wide slices + 4 in cx-tag tiles)."""
                scg_a = ps.tile([128, 1024], F32, name=f"scga_{name}", tag="sc", bufs=2)
                scg_b = ps.tile([128, 1024], F32, name=f"scgb_{name}", tag="sc", bufs=2)
                scg = (scg_a, scg_b)
                ctxg = [
                    ps.tile([128, 512], F32, name=f"cg_{name}{g}", tag="cx", bufs=4)
                    for g in range(4)
                ]
                xts, ws = [], []
                for d in range(ND):
                    xv = jp.tile(
                        [128, S], F32R, name=f"x_{name}{d}", tag="xt", bufs=8
                    )
                    nc.sync.dma_start(out=xv, in_=xt_dram[128 * d : 128 * (d + 1), :])
                    xts.append(xv)
                    wv = jp.tile(
                        [128, wv_width], F32R, name=f"w_{name}{d}", tag="wp", bufs=6
                    )
                    nc.sync.dma_start(out=wv, in_=w_dram[128 * d : 128 * (d + 1), :])
                    ws.append(wv)
                for d in range(ND):
                    emit_mms(d, xts[d], ws[d], scg, ctxg)
                emit_evacs(scg, ctxg)

            def v_mms(d, xv, wv, scg, ctxg):
                for s in range(NS):
                    dst = (
                        scg[s // 2][:, 512 * (s % 2) : 512 * (s % 2) + VW]
                        if s < 4
                        else ctxg[s - 4][:, 0:VW]
                    )
                    nc.tensor.matmul(
                        dst,
                        xv[:, 128 * s : 128 * (s + 1)],
                        wv,
                        start=(d == 0),
                        stop=(d == ND - 1),
                    )

            def v_evacs(scg, ctxg):
                # copy only the 16 data columns of each 17-wide head slot
                # (ones columns pre-set); ACT engine is idle in this phase
                for s in range(NS):
                    src = (
                        scg[s // 2][:, 512 * (s % 2) : 512 * (s % 2) + VW]
                        if s < 4
                        else ctxg[s - 4][:, 0:VW]
                    )
                    src3 = src.rearrange("p (m c) -> p m c", c=17)
                    dst3 = va_sb[s][:, 0:VW].rearrange("p (m c) -> p m c", c=17)
                    # DVE (not ACT): V runs last, concurrent with attention exps
                    nc.vector.tensor_copy(dst3[:, :, 0:16], src3[:, :, 0:16])

            def qk_mms_factory(which):
                def mms(d, xv, wv, scg, ctxg):
                    for t in range(NQUAD):
                        for h in range(2):
                            g = 2 * t + h
                            dst = (
                                scg[g // 2][:, 512 * (g % 2) : 512 * (g % 2 + 1)]
                                if g < 4
                                else ctxg[g - 4]
                            )
                            nc.tensor.matmul(
                                dst,
                                wv[:, 128 * t : 128 * (t + 1)],
                                xv[:, 512 * h : 512 * (h + 1)],
                                start=(d == 0),
                                stop=(d == ND - 1),
                            )

                return mms

            def qk_evacs_factory(dst_tiles, bias):
                def evacs(scg, ctxg):
                    for t in range(NQUAD):
                        for h in range(2):
                            g = 2 * t + h
                            src = (
                                scg[g // 2][:, 512 * (g % 2) : 512 * (g % 2 + 1)]
                                if g < 4
                                else ctxg[g - 4]
                            )
                            nc.scalar.activation(
                                dst_tiles[t][:, 512 * h : 512 * (h + 1)],
                                src,
                                AF.Identity,
                                bias=bias[:, t : t + 1],
                            )

                return evacs

            stream_proj(xt_q, wqt, 512, qk_mms_factory("q"), qk_evacs_factory(qt_sb, bq_sb), "q")
            stream_proj(xt_k, wkt, 512, qk_mms_factory("k"), qk_evacs_factory(kt_sb, bk_sb), "k")
            stream_proj(xt_v, wvt, VW, v_mms, v_evacs, "v")

            for t in range(NQUAD):
                nc.sync.dma_start(out=wot_sb[t], in_=wot[128 * t : 128 * (t + 1), :])

            # ================= attention (n-outer) =================
            def outproj_group(m, dc, evac_act=False):
                po = ps.tile([128, 512], F32, name=f"po{m}{dc}", tag="cx", bufs=4)
                for t_ in range(NQUAD):
                    nc.tensor.matmul(
                        po,
                        ctxp[t_][:, 128 * m : 128 * (m + 1)],
                        wot_sb[t_][:, 512 * dc : 512 * (dc + 1)],
                        start=(t_ == 0),
                        stop=(t_ == NQUAD - 1),
                    )
                og = ap_.tile([128, 512], F32, name=f"og{m}{dc}", tag="og", bufs=4)
                if evac_act:
                    nc.scalar.copy(og, po)
                else:
                    nc.vector.tensor_copy(og, po)
                nc.sync.dma_start(
                    out=out_d[128 * m : 128 * (m + 1), 512 * dc : 512 * (dc + 1)],
                    in_=og,
                )

            for n in range(2):  # sq chunks of 512
                stages = {}
                den_h = {}
                for t in range(NQUAD):
                    if t % 2 == 0:
                        den_h[t // 2] = ap_.tile(
                            [8, 512], F32, name=f"den{n}{t//2}", tag="den", bufs=2
                        )
                    den_n = den_h[t // 2]
                    ctx_t = [
                        ps.tile([17, 512], F32, name=f"ctx{t}{n}{j}", tag="cx", bufs=4)
                        for j in range(NHQ)
                    ]
                    ex_tiles = []
                    for i in range(NS):  # sk blocks of 128
                        sc_a = ps.tile(
                            [128, 1024], F32, name=f"sca{t}{n}{i}", tag="sc", bufs=2
                        )
                        sc_b = ps.tile(
                            [128, 1024], F32, name=f"scb{t}{n}{i}", tag="sc", bufs=2
                        )
                        ex = ap_.tile(
                            [128, 2048], F32R, name=f"ex{t}{n}{i}", tag="ex", bufs=3
                        )
                        for j in range(NHQ):
                            sch = sc_a if j < 2 else sc_b
                            nc.tensor.matmul(
                                sch[:, 512 * (j % 2) : 512 * (j % 2 + 1)],
                                kt_sb[t][32 * j : 32 * j + 32, 128 * i : 128 * (i + 1)],
                                qt_sb[t][32 * j : 32 * j + 32, 512 * n : 512 * (n + 1)],
                                start=True,
                                stop=True,
                                tile_position=(32 * j, 0),
                            )
                        nc.scalar.activation(ex[:, 0:1024], sc_a, AF.Exp, scale=SCALE)
                        nc.scalar.activation(
                            ex[:, 1024:2048], sc_b, AF.Exp, scale=SCALE
                        )
                        ex_tiles.append(ex)
                        if i >= 1:
                            for j in range(NHQ):
                                mq = 17 * (NHQ * t + j)
                                nc.tensor.matmul(
                                    ctx_t[j],
                                    va_sb[i - 1][:, mq : mq + 17],
                                    ex_tiles[i - 1][:, 512 * j : 512 * (j + 1)],
                                    start=(i - 1 == 0),
                                    stop=False,
                                )
                    for j in range(NHQ):
                        mq = 17 * (NHQ * t + j)
                        nc.tensor.matmul(
                            ctx_t[j],
                            va_sb[NS - 1][:, mq : mq + 17],
                            ex_tiles[NS - 1][:, 512 * j : 512 * (j + 1)],
                            start=False,
                            stop=True,
                        )

                    # stage ctx + gather denominators (ones rows at 32j+16)
                    st = ap_.tile(
                        [128, 512], F32, name=f"st{t}{n}", tag="stage", bufs=6
                    )
                    for j in range(NHQ):
                        nc.vector.tensor_copy(
                            st[32 * j : 32 * j + 17, :], ctx_t[j][0:17, :]
                        )
                        m8 = NHQ * (t % 2) + j
                        nc.sync.dma_start(
                            out=den_n[m8 : m8 + 1, :],
                            in_=st[32 * j + 16 : 32 * j + 17, :],
                        )
                    stages[t] = st

                    # interleave first-half output projections into n=1
                    if n == 1 and t >= 2:
                        for m_ in (2 * (t - 2), 2 * (t - 2) + 1):
                            for dc in range(2):
                                outproj_group(m_, dc)

                    if n == 1 and t == 3:
                        # PE warm-up burst overlapping the final normalize's
                        # DVE chain, so the tail outproj runs at full clock
                        dum_ps = ps.tile(
                            [128, 512], F32, name="dum_ps", tag="sc", bufs=2
                        )
                        for w_i in range(18):
                            nc.tensor.matmul(
                                dum_ps,
                                kt_sb[w_i % 4][:, 0:128],
                                qt_sb[w_i % 4][:, 0:512],
                                start=True,
                                stop=True,
                                skip_group_check=True,
                            )
                        nc.vector.tensor_copy(dummy, dum_ps[0:1, 0:8])

                    if t % 2 == 1:
                        # normalize the half's 8 heads
                        h = t // 2
                        with tc.high_priority(offset=-160):
                            recip = ap_.tile(
                                [8, 512], F32, name=f"rc{n}{h}", tag="recip", bufs=2
                            )
                            scratch = ap_.tile(
                                [8, 512], F32, name=f"rs{n}{h}", tag="recip", bufs=2
                            )
                            nc.vector.reciprocal_approx_accurate(
                                recip, den_h[h], scratch
                            )
                            recipr = ap_.tile(
                                [8, 512], F32R, name=f"rr{n}{h}", tag="recipr", bufs=2
                            )
                            nc.vector.tensor_copy(recipr, recip)
                            for t_ in (2 * h, 2 * h + 1):
                                rbw = ps.tile(
                                    [128, 512], F32, name=f"rbw{n}{t_}", tag="cx",
                                    bufs=4,
                                )
                                nc.tensor.matmul(
                                    rbw,
                                    ind_sb[:, 128 * (t_ % 2) : 128 * (t_ % 2 + 1)],
                                    recipr,
                                    start=True,
                                    stop=True,
                                )
                                # evacuate the PSUM bank fast, then normalize
                                # on the (otherwise idle) GPSIMD engine
                                rb = ap_.tile(
                                    [128, 512], F32, name=f"rb{n}{t_}", tag="rb",
                                    bufs=2,
                                )
                                nc.vector.tensor_copy(rb, rbw)
                                eng = nc.gpsimd if n == 0 else nc.vector
                                for j in range(NHQ):
                                    eng.tensor_tensor(
                                        ctxp[t_][
                                            32 * j : 32 * j + 16,
                                            512 * n : 512 * (n + 1),
                                        ],
                                        rb[32 * j : 32 * j + 16, :],
                                        stages[t_][32 * j : 32 * j + 16, :],
                                        ALU.mult,
                                    )

            # remaining output projections (second sq half)
            for m in range(4, 8):
                for dc in range(2):
                    outproj_group(m, dc, evac_act=True)

    nc.finalize()
    return nc


def prep_core_weights(g, Wq, bq, Wk, bk, Wv, Wo):
    C0 = 256 * g
    wqt = np.zeros((D, 512), np.float32)
    wkt = np.zeros((D, 512), np.float32)
    wvt = np.zeros((D, VW), np.float32)
    wot = np.zeros((512, D), np.float32)
    bqp = np.zeros((128, NQUAD), np.float32)
    bkp = np.zeros((128, NQUAD), np.float32)
    for t in range(NQUAD):
        for j in range(NHQ):
            src = C0 + 64 * t + 16 * j
            wqt[:, 128 * t + 32 * j : 128 * t + 32 * j + E] = Wq[src : src + E, :].T
            wkt[:, 128 * t + 32 * j : 128 * t + 32 * j + E] = Wk[src : src + E, :].T
            bqp[32 * j : 32 * j + E, t] = bq[src : src + E]
            bkp[32 * j : 32 * j + E, t] = bk[src : src + E]
            m = NHQ * t + j
            wvt[:, 17 * m : 17 * m + E] = Wv[src : src + E, :].T
            wot[128 * t + 32 * j : 128 * t + 32 * j + E, :] = Wo[:, src : src + E].T
    ind_np = np.zeros((8, 256), np.float32)
    for m8 in range(8):
        ind_np[m8, 128 * (m8 // 4) + 32 * (m8 % 4) : 128 * (m8 // 4) + 32 * (m8 % 4) + E] = 1.0
    return {
        "wqt": round_fp32r(wqt),
        "wkt": round_fp32r(wkt),
        "wvt": round_fp32r(wvt),
        "wot": round_fp32r(wot),
        "bqp": bqp,
        "bkp": bkp,
        "ind_all": round_fp32r(ind_np),
    }


def prep_in_maps(Q, K, V, Wq, bq, Wk, bk, Wv, Wo):
    group_w = [prep_core_weights(g, Wq, bq, Wk, bk, Wv, Wo) for g in range(4)]
    xt = []
    for b in range(B):
        xt.append(
            {
                "xt_q": round_fp32r(np.ascontiguousarray(Q[b].T)),
                "xt_k": round_fp32r(np.ascontiguousarray(K[b].T)),
                "xt_v": round_fp32r(np.ascontiguousarray(V[b].T)),
            }
        )
    in_maps = []
    for c in range(8):
        b, g = c // 4, c % 4
        m = dict(group_w[g])
        m.update(xt[b])
        in_maps.append(m)
    return in_maps


def assemble_output(results, bv, bo, Wo):
    # v-bias is folded here: sum_k attn = 1 exactly, so the missing
    # (v + bv) contribution is the constant vector Wo @ bv per position.
    const = (
        Wo.astype(np.float64) @ bv.astype(np.float64) + bo.astype(np.float64)
    )
    out = np.zeros((B, S, D), np.float32)
    for b in range(B):
        acc = np.zeros((S, D), np.float64)
        for g in range(4):
            acc += results[4 * b + g]["out_part"].astype(np.float64)
        out[b] = (acc + const).astype(np.float32)
    return out


_NC_CACHE = {}


def _get_nc():
    if "nc" not in _NC_CACHE:
        _NC_CACHE["nc"] = build_nc()
    return _NC_CACHE["nc"]


def kernel(Q, K, V, Wq, bq, Wk, bk, Wv, bv, Wo, bo):
    import time

    from concourse.bass_utils import run_bass_kernel_spmd

    nc = _get_nc()
    in_maps = prep_in_maps(
        np.asarray(Q, np.float32),
        np.asarray(K, np.float32),
        np.asarray(V, np.float32),
        np.asarray(Wq, np.float32),
        np.asarray(bq, np.float32),
        np.asarray(Wk, np.float32),
        np.asarray(bk, np.float32),
        np.asarray(Wv, np.float32),
        np.asarray(Wo, np.float32),
    )
    # Retries: a first execution after NEFF load occasionally hits a
    # transient NRT_EXEC_UNIT_UNRECOVERABLE; re-running recovers.
    last = None
    for attempt in range(3):
        try:
            res = run_bass_kernel_spmd(nc, in_maps, list(range(8)))
            return assemble_output(
                res.results,
                np.asarray(bv, np.float32),
                np.asarray(bo, np.float32),
                np.asarray(Wo, np.float32),
            )
        except Exception as e:
            last = e
            time.sleep(3)
    raise last
